# revision 8
# baseline (speedup 1.0000x reference)
"""DeepWuKong GCN (3-layer GCNConv + max/mean pool + FFN) on 8 TRN2 NeuronCores.

v3 — deep-prefetch pipeline, merged SWDGE calls:
  - Layout as v2: 16 graphs/core, 512 slots each (64 blocks of 128); z table
    rows hold dis[s]*z[s] fp16; two 32768-row AllGather halves per layer;
    per-128-edge-chunk mask matmuls aggregate into [feat, dst] PSUM.
  - Gather calls merged over groups of 2 dst blocks (one SWDGE call covers
    both blocks' chunks; all pad slots are idx-0 descriptors so every output
    slot is written -> no memset, no trailing -1 bookkeeping).
  - Deep prefetch: lo gathers lead consumption by 6 groups, hi by 4 groups,
    mask streams by 6 blocks. Emission order keeps the in-order GpSimd queue
    from head-blocking on the not-yet-AllGathered hi table at layer start.
  - AllGather lo-half fires at consume step 31 (data ready), hi after 63;
    transform z-copy runs on DVE; pooling reduces interleave into the last
    layer's consume loop.
"""
import sys

sys.path.insert(0, "/opt/trn_rl_repo")

import numpy as np

import concourse.bacc as bacc
import concourse.bass as bass
import concourse.mybir as mybir
import concourse.tile as tile
from concourse.bass_utils import run_bass_kernel_spmd

# ---- problem constants (hardcoded per spec) --------------------------------
N_NODES = 50000
N_EDGES = 600000
N_GRAPHS = 128
D = 128
N_LAYERS = 3
N_CORES = 8
GPC = N_GRAPHS // N_CORES      # 16 graphs per core
GSLOT = 512                    # node slots per graph (4 blocks of 128)
NLOC = GPC * GSLOT             # 8192 node slots per core
NBLK = NLOC // 128             # 64 blocks per core
BPG = GSLOT // 128             # blocks per graph
TOT = N_CORES * NLOC           # 65536 table rows
SPLIT = 32768                  # int16 gather index split

F32 = mybir.dt.float32
F16 = mybir.dt.float16
I16 = mybir.dt.int16

DMA_SCRATCH = 32768
N_QUEUES = 4
GM = 2                         # dst blocks merged per gather call
MAX_CALL_CHUNKS = 8            # <= 1024 idx per call (ucode limit suspected)
LEAD_LO_G = 6                  # lo gather lead, in groups
LEAD_HI_G = 4                  # hi gather lead, in groups
LEAD_MK = 6                    # mask prefetch lead, in blocks


# ===========================================================================
# host-side schedule construction
# ===========================================================================
def _build_schedule(x, edge_index, batch):
    x = np.asarray(x, np.float32)
    ei = np.asarray(edge_index).astype(np.int64)
    batch = np.asarray(batch).astype(np.int64)

    counts = np.bincount(batch, minlength=N_GRAPHS)
    assert counts.max() <= GSLOT, f"graph too big: {counts.max()}"

    deg = np.bincount(ei[1], minlength=N_NODES).astype(np.float64) + 1.0
    dis = 1.0 / np.sqrt(deg)

    graph_start = np.zeros(N_GRAPHS + 1, np.int64)
    np.cumsum(counts, out=graph_start[1:])

    # degree-balanced placement of each graph's nodes into its BPG blocks
    newslot = np.full(N_NODES, -1, np.int64)   # global slot: core*NLOC + b*128 + p
    for g in range(N_GRAPHS):
        nodes = np.arange(graph_start[g], graph_start[g + 1])
        if len(nodes) == 0:
            continue
        order = np.argsort(-deg[nodes], kind="stable")
        base = (g // GPC) * NLOC + (g % GPC) * GSLOT
        bin_load = np.zeros(BPG)
        bin_fill = np.zeros(BPG, np.int64)
        for n in nodes[order]:
            cand = np.argsort(bin_load, kind="stable")
            for b in cand:
                if bin_fill[b] < 128:
                    break
            newslot[n] = base + b * 128 + bin_fill[b]
            bin_fill[b] += 1
            bin_load[b] += deg[n]
    assert (newslot >= 0).all()

    core_of = newslot // NLOC
    lslot = newslot % NLOC
    blk_of = lslot // 128          # local block 0..63
    part_of = lslot % 128
    # table rows: 2 AllGather halves of 32 blocks; within a half the AG
    # concatenates ranks, each rank contributing [128 p, 32 b] node-major
    q_of = blk_of // 32
    row_of = q_of * SPLIT + core_of * 4096 + part_of * 32 + (blk_of % 32)

    # real edges only (self loops injected on device via diag matmul)
    src, dst = ei[0], ei[1]
    srow = row_of[src]
    dcore = core_of[dst]
    dblk = blk_of[dst]
    dpart = part_of[dst]
    hi = (srow >= SPLIT).astype(np.int64)

    # per (core, blk, bucket) counts -> shared K across cores
    cnt = np.zeros((N_CORES, NBLK, 2), np.int64)
    np.add.at(cnt, (dcore, dblk, hi), 1)
    Vmax = cnt.max(axis=0)                      # [NBLK, 2]
    K = -(-Vmax // 128)                         # chunks (may be 0)
    K_lo = K[:, 0].astype(int)
    K_hi = K[:, 1].astype(int)

    lo_off = np.zeros(NBLK + 1, np.int64)
    np.cumsum(K_lo * 128, out=lo_off[1:])
    hi_off = np.zeros(NBLK + 1, np.int64)
    np.cumsum(K_hi * 128, out=hi_off[1:])
    ch_off = np.zeros(NBLK + 1, np.int64)
    np.cumsum(K_lo + K_hi, out=ch_off[1:])
    nlo_slots = int(lo_off[-1])
    nhi_slots = int(hi_off[-1])
    NCH = int(ch_off[-1])

    # all pad slots are idx 0 (real row, zero mask) so every call processes
    # its full K*128 slots and every output slot is DMA-written.
    idx_lo = np.zeros((N_CORES, max(nlo_slots, 16)), np.int16)
    idx_hi = np.zeros((N_CORES, max(nhi_slots, 16)), np.int16)

    # vectorized per-(core,blk,bucket) slot assignment
    sort = np.lexsort((hi, dblk, dcore))
    s_core, s_blk, s_hi = dcore[sort], dblk[sort], hi[sort]
    s_row, s_dp = srow[sort], dpart[sort]
    gid = (s_core * NBLK + s_blk) * 2 + s_hi
    first = np.ones(len(gid), bool)
    first[1:] = gid[1:] != gid[:-1]
    gstart = np.zeros(len(gid), np.int64)
    idxs_first = np.flatnonzero(first)
    gstart[idxs_first] = idxs_first
    gstart = np.maximum.accumulate(gstart)
    pos = np.arange(len(gid)) - gstart

    slot = np.where(s_hi == 0, lo_off[s_blk], hi_off[s_blk]) + pos
    chcol = np.where(s_hi == 0, ch_off[s_blk], ch_off[s_blk] + K_lo[s_blk]) \
        + pos // 128
    val = np.where(s_hi == 0, s_row, s_row - SPLIT).astype(np.int16)
    lom = s_hi == 0
    idx_lo[s_core[lom], slot[lom]] = val[lom]
    idx_hi[s_core[~lom], slot[~lom]] = val[~lom]

    def wrap_idx(a):                 # [slots] -> [128, slots/16], 8x replicated
        pad = (-len(a)) % 16
        if pad:
            a = np.concatenate([a, np.zeros(pad, np.int16)])
        w16 = a.reshape(-1, 16).T
        return np.tile(w16, (8, 1)).copy()

    idx_lo_w = np.stack([wrap_idx(idx_lo[c]) for c in range(N_CORES)])
    idx_hi_w = np.stack([wrap_idx(idx_hi[c]) for c in range(N_CORES)])

    # feature-major x per core, fp16, columns ordered by slot (b*128+p)
    xpad = np.zeros((N_CORES * NLOC, D), np.float32)
    xpad[newslot] = x
    x_fm = np.stack([xpad[c * NLOC:(c + 1) * NLOC].T.copy()
                     for c in range(N_CORES)]).astype(np.float16)

    # host-built mask tiles scaled by dis[d]: [core][128 edge, NCH*128]
    nch = max(NCH, 1)
    ohmask = np.zeros((N_CORES, 128, nch, 128), np.float16)
    cc, pp, hh = s_core, pos % 128, chcol
    ohmask[cc, pp, hh, s_dp] = dis[dst[sort]].astype(np.float16)
    ohmask = ohmask.reshape(N_CORES, 128, nch * 128)

    # per-slot dis, node-major: discol[c][p, b] = dis(node at (c,b,p))
    discol = np.zeros((N_CORES, 128, NBLK), np.float32)
    discol[core_of, part_of, blk_of] = dis.astype(np.float32)
    # self-loop diag tiles: diagdis[c][p, b*128+d] = (p==d) * dis(c,b,d)
    diagdis = np.zeros((N_CORES, 128, NBLK, 128), np.float16)
    diagdis[core_of, part_of, blk_of, part_of] = dis.astype(np.float16)
    diagdis = diagdis.reshape(N_CORES, 128, NBLK * 128)

    invcnt = (1.0 / np.maximum(counts, 1)).astype(np.float32)
    invcnt_rep = np.stack([
        np.tile(invcnt[c * GPC:(c + 1) * GPC], (128, 1)) for c in range(N_CORES)
    ]).astype(np.float32)

    return dict(
        K_lo=K_lo, K_hi=K_hi, NCH=NCH,
        nlo16=idx_lo_w.shape[2], nhi16=idx_hi_w.shape[2],
        lo_off=lo_off, hi_off=hi_off, ch_off=ch_off,
        idx_lo=idx_lo_w, idx_hi=idx_hi_w,
        ohmask=ohmask, discol=discol, diagdis=diagdis,
        x_fm=x_fm, invcnt_rep=invcnt_rep,
    )


# ===========================================================================
# device kernel
# ===========================================================================
def _build_kernel(sch):
    K_lo, K_hi = sch["K_lo"], sch["K_hi"]
    lo_off, hi_off, ch_off = sch["lo_off"], sch["hi_off"], sch["ch_off"]
    NCH = max(sch["NCH"], 1)
    NLO16 = sch["nlo16"]
    NHI16 = sch["nhi16"]
    NG = NBLK // GM

    # per-group chunk counts / offsets; split a group call if it exceeds the
    # descriptor-ring budget (K arrays are cross-core shared, so SPMD-safe)
    def group_calls(Koff, Karr):
        calls = []                  # per group: list of (off16, nchunks)
        for g in range(NG):
            b0 = g * GM
            ks = [int(Karr[b]) for b in range(b0, b0 + GM)]
            if sum(ks) <= MAX_CALL_CHUNKS:
                if sum(ks):
                    calls.append([(int(Koff[b0]) // 16, sum(ks))])
                else:
                    calls.append([])
            else:
                calls.append([(int(Koff[b]) // 16, k)
                              for b, k in zip(range(b0, b0 + GM), ks) if k])
        return calls

    lo_calls = group_calls(lo_off, K_lo)
    hi_calls = group_calls(hi_off, K_hi)

    nc = bacc.Bacc(
        "TRN2",
        target_bir_lowering=False,
        debug=False,
        num_devices=N_CORES,
        num_swdge_queues=N_QUEUES,
        dynamic_dma_scratch_size=DMA_SCRATCH,
    )

    xfm_d = nc.dram_tensor("xfm", [128, NLOC], F16, kind="ExternalInput")
    wc_d = nc.dram_tensor("wc", [N_LAYERS, 128, 128], F16, kind="ExternalInput")
    bct_d = nc.dram_tensor("bct", [128, N_LAYERS], F16, kind="ExternalInput")
    wffn_d = nc.dram_tensor("wffn", [256, 128], F32, kind="ExternalInput")
    bffnt_d = nc.dram_tensor("bffnt", [128, 1], F32, kind="ExternalInput")
    wfin_d = nc.dram_tensor("wfin", [128, 2], F32, kind="ExternalInput")
    bfinr_d = nc.dram_tensor("bfinr", [GPC, 2], F32, kind="ExternalInput")
    idxlo_d = nc.dram_tensor("idxlo", [128, NLO16], I16, kind="ExternalInput")
    idxhi_d = nc.dram_tensor("idxhi", [128, NHI16], I16, kind="ExternalInput")
    ohmask_d = nc.dram_tensor("ohmask", [128, NCH * 128], F16,
                              kind="ExternalInput")
    discol_d = nc.dram_tensor("discol", [128, NBLK], F32, kind="ExternalInput")
    diagdis_d = nc.dram_tensor("diagdis", [128, NBLK * 128], F16,
                               kind="ExternalInput")
    invc_d = nc.dram_tensor("invc", [128, GPC], F32, kind="ExternalInput")
    out_d = nc.dram_tensor("out", [GPC, 2], F32, kind="ExternalOutput")

    RG = [list(range(N_CORES))]

    with tile.TileContext(nc) as tc:
        with (
            tc.tile_pool(name="consts", bufs=1) as consts,
            tc.tile_pool(name="hpool", bufs=2) as hpool,
            tc.tile_pool(name="zpool", bufs=2) as zpool,
            tc.tile_pool(name="gpool", bufs=LEAD_LO_G + 2) as gpool,
            tc.tile_pool(name="gpool2", bufs=LEAD_HI_G + 2) as gpool2,
            tc.tile_pool(name="ohpool", bufs=LEAD_MK + 2) as ohpool,
            tc.tile_pool(name="spool", bufs=1) as spool,
            tc.tile_pool(name="ps128", bufs=2, space="PSUM") as ps128,
            tc.tile_pool(name="psagg", bufs=4, space="PSUM") as psagg,
            tc.tile_pool(name="psfin", bufs=1, space="PSUM") as psfin,
            tc.tile_pool(name="dram", bufs=1, space="DRAM") as dram,
        ):
            # ---- load constants (critical-path inputs first) --------------
            h_cur0 = hpool.tile([128, NLOC], F16, tag="h", name="h_init")
            nc.sync.dma_start(h_cur0[:], xfm_d[:])
            wc_sb = consts.tile([128, N_LAYERS, 128], F16)
            nc.sync.dma_start(wc_sb[:], wc_d[:].rearrange("l p f -> p l f"))
            discol_sb = consts.tile([128, NBLK], F32)
            nc.sync.dma_start(discol_sb[:], discol_d[:])
            idxlo_sb = consts.tile([128, NLO16], I16)
            nc.sync.dma_start(idxlo_sb[:], idxlo_d[:])
            idxhi_sb = consts.tile([128, NHI16], I16)
            nc.sync.dma_start(idxhi_sb[:], idxhi_d[:])
            diagdis_sb = consts.tile([128, NBLK, 128], F16)
            nc.sync.dma_start(
                diagdis_sb[:], diagdis_d[:].rearrange("p (b d) -> p b d", d=128))
            bct_sb = consts.tile([128, N_LAYERS], F16)
            nc.sync.dma_start(bct_sb[:], bct_d[:])
            wffn_sb = consts.tile([128, 2, 128], F32)
            nc.sync.dma_start(
                wffn_sb[:], wffn_d[:].rearrange("(h p) f -> p h f", p=128))
            bffnt_sb = consts.tile([128, 1], F32)
            nc.sync.dma_start(bffnt_sb[:], bffnt_d[:])
            wfin_sb = consts.tile([128, 2], F32)
            nc.sync.dma_start(wfin_sb[:], wfin_d[:])
            bfinr_sb = consts.tile([GPC, 2], F32)
            nc.sync.dma_start(bfinr_sb[:], bfinr_d[:])
            invc_sb = consts.tile([128, GPC], F32)
            nc.sync.dma_start(invc_sb[:], invc_d[:])

            def transform_blk(l, h_src, z_nm, b):
                zps = ps128.tile([128, 128], F32, tag="zps",
                                 name=f"zps{l}_{b}")
                nc.tensor.matmul(
                    zps[:], h_src[:, b * 128:(b + 1) * 128],
                    wc_sb[:, l, :], start=True, stop=True)
                nc.vector.tensor_scalar_mul(
                    z_nm[:, b, :], zps[:], discol_sb[:, b:b + 1])

            def share_half(l, z_nm, q):
                z_own = dram.tile([128, 32, 128], F16, tag=f"zown{q}",
                                  bufs=2, name=f"zown{l}_{q}")
                nc.sync.dma_start(
                    z_own[:], z_nm[:, q * 32:(q + 1) * 32, :])
                z_half = dram.tile([SPLIT, 128], F16, tag=f"zfull{q}",
                                   bufs=2, addr_space="Shared",
                                   name=f"zfull{l}_{q}")
                nc.gpsimd.collective_compute(
                    "AllGather", mybir.AluOpType.bypass,
                    replica_groups=RG,
                    ins=[z_own[:].opt()],
                    outs=[z_half[:].opt()],
                )
                return z_half

            qc = [0]

            def emit_calls(pool, tag, l, g, calls, table, idx_sb):
                tiles = []
                for ci, (off16, kg) in enumerate(calls[g]):
                    gt = pool.tile([128, kg, 128], F16, tag=tag,
                                   name=f"{tag}{l}_{g}_{ci}")
                    nc.gpsimd.dma_gather(
                        gt[:], table[:],
                        idx_sb[:, off16:off16 + kg * 8],
                        num_idxs=kg * 128, num_idxs_reg=kg * 128,
                        elem_size=128, queue_num=qc[0] % N_QUEUES,
                    )
                    qc[0] += 1
                    tiles.append(gt)
                return tiles

            # pooling accumulators (columns filled during the last layer)
            mx = spool.tile([128, GPC], F32)
            sm = spool.tile([128, GPC], F32)

            # ---- layer 0 transform + first tables -------------------------
            z_nm = zpool.tile([128, NBLK, 128], F16, tag="znm", name="znm0")
            z_lo0 = None
            for b in range(NBLK):
                transform_blk(0, h_cur0, z_nm, b)
                if b == 31:
                    z_lo0 = share_half(0, z_nm, 0)
            z_full = [z_lo0, share_half(0, z_nm, 1)]
            h_cur = h_cur0

            for l in range(N_LAYERS):
                h_nxt = hpool.tile([128, NLOC], F16, tag="h", name=f"h{l + 1}")
                z_nm_nxt = None
                if l + 1 < N_LAYERS:
                    z_nm_nxt = zpool.tile([128, NBLK, 128], F16, tag="znm",
                                          name=f"znm{l + 1}")
                z_lo_nxt = None

                glo_t = {}     # group -> list of lo tiles
                ghi_t = {}
                mk_t = {}      # block -> mask tile

                def emit_lo(g):
                    glo_t[g] = emit_calls(gpool, "glo", l, g, lo_calls,
                                          z_full[0], idxlo_sb)

                def emit_hi(g):
                    ghi_t[g] = emit_calls(gpool2, "ghi", l, g, hi_calls,
                                          z_full[1], idxhi_sb)

                def emit_mask(b):
                    ktot = int(K_lo[b] + K_hi[b])
                    if not ktot:
                        mk_t[b] = None
                        return
                    ch0 = int(ch_off[b])
                    mkt = ohpool.tile([128, ktot * 128], F16,
                                      tag="oh", name=f"oh{l}_{b}")
                    nc.sync.dma_start(
                        mkt[:], ohmask_d[:, ch0 * 128:(ch0 + ktot) * 128])
                    mk_t[b] = mkt

                # preamble: all lo leads BEFORE any hi (hi waits on the
                # just-issued AllGather; keep it off the GpSimd queue head)
                for g in range(min(LEAD_LO_G, NG)):
                    emit_lo(g)
                for b in range(min(LEAD_MK, NBLK)):
                    emit_mask(b)
                for g in range(min(LEAD_HI_G, NG)):
                    emit_hi(g)

                for s in range(NBLK):
                    if s % GM == 0:
                        g = s // GM
                        if g + LEAD_LO_G < NG:
                            emit_lo(g + LEAD_LO_G)
                        if g + LEAD_HI_G < NG:
                            emit_hi(g + LEAD_HI_G)
                    if s + LEAD_MK < NBLK:
                        emit_mask(s + LEAD_MK)

                    # ---- consume block s -----------------------------------
                    b = s
                    g = b // GM
                    klo, khi = int(K_lo[b]), int(K_hi[b])
                    ktot = klo + khi
                    # chunk offset of block b within its group's tiles
                    ofs_lo = sum(int(K_lo[bb]) for bb in range(g * GM, b))
                    ofs_hi = sum(int(K_hi[bb]) for bb in range(g * GM, b))

                    def chunk_ap(tiles, ofs, j):
                        # walk the (possibly split) call tiles of the group
                        for t in tiles:
                            n = t.shape[1]
                            if ofs + j < n:
                                return t[:, ofs + j, :]
                            ofs -= n
                        raise AssertionError("chunk out of range")

                    ps = psagg.tile([128, 128], F32, tag="aggps",
                                    name=f"agg{l}_{b}")
                    nc.tensor.matmul(
                        ps[:], z_nm[:, b, :], diagdis_sb[:, b, :],
                        start=True, stop=(ktot == 0))
                    if ktot:
                        mk = mk_t.pop(b)[:]
                        for jj in range(ktot):
                            msg = chunk_ap(glo_t[g], ofs_lo, jj) if jj < klo \
                                else chunk_ap(ghi_t[g], ofs_hi, jj - klo)
                            nc.tensor.matmul(
                                ps[:], msg, mk[:, jj * 128:(jj + 1) * 128],
                                start=False, stop=(jj == ktot - 1))
                    nc.scalar.activation(
                        h_nxt[:, b * 128:(b + 1) * 128], ps[:],
                        mybir.ActivationFunctionType.Relu,
                        bias=bct_sb[:, l:l + 1])

                    if z_nm_nxt is not None:
                        transform_blk(l + 1, h_nxt, z_nm_nxt, b)
                        if b == 31:
                            z_lo_nxt = share_half(l + 1, z_nm_nxt, 0)

                    if l == N_LAYERS - 1 and b % BPG == BPG - 1:
                        gi = b // BPG
                        nc.vector.tensor_reduce(
                            mx[:, gi:gi + 1],
                            h_nxt[:, gi * GSLOT:(gi + 1) * GSLOT],
                            mybir.AxisListType.X, mybir.AluOpType.max)
                        nc.vector.tensor_reduce(
                            sm[:, gi:gi + 1],
                            h_nxt[:, gi * GSLOT:(gi + 1) * GSLOT],
                            mybir.AxisListType.X, mybir.AluOpType.add)

                if z_nm_nxt is not None:
                    z_nm = z_nm_nxt
                    z_full = [z_lo_nxt, share_half(l + 1, z_nm_nxt, 1)]
                h_cur = h_nxt

            # ---- pooling + FFN --------------------------------------------
            mean = spool.tile([128, GPC], F32)
            nc.vector.tensor_tensor(
                mean[:], sm[:], invc_sb[:], mybir.AluOpType.mult)

            p1 = psfin.tile([128, GPC], F32, tag="p1")
            nc.tensor.matmul(p1[:], wffn_sb[:, 0, :], mx[:],
                             start=True, stop=False)
            nc.tensor.matmul(p1[:], wffn_sb[:, 1, :], mean[:],
                             start=False, stop=True)
            o1 = spool.tile([128, GPC], F32)
            nc.scalar.activation(
                o1[:], p1[:], mybir.ActivationFunctionType.Relu,
                bias=bffnt_sb[:, 0:1])

            p2 = psfin.tile([GPC, 2], F32, tag="p2")
            nc.tensor.matmul(p2[:], o1[:], wfin_sb[:], start=True, stop=True)
            osb = spool.tile([GPC, 2], F32)
            nc.vector.tensor_tensor(
                osb[:], p2[:], bfinr_sb[:], mybir.AluOpType.add)
            nc.sync.dma_start(out_d[:], osb[:])

    nc.compile()
    return nc


# ===========================================================================
# entry point
# ===========================================================================
_CACHE = {}


def kernel(x, Wc, bc, W_ffn, b_ffn, W_fin, b_fin, edge_index, batch):
    x = np.ascontiguousarray(np.asarray(x, np.float32))
    Wc = np.ascontiguousarray(np.asarray(Wc, np.float32))
    bc = np.ascontiguousarray(np.asarray(bc, np.float32))
    W_ffn = np.ascontiguousarray(np.asarray(W_ffn, np.float32))
    b_ffn = np.ascontiguousarray(np.asarray(b_ffn, np.float32))
    W_fin = np.ascontiguousarray(np.asarray(W_fin, np.float32))
    b_fin = np.ascontiguousarray(np.asarray(b_fin, np.float32))

    sch = _build_schedule(x, edge_index, batch)

    key = (sch["NCH"], sch["nlo16"], sch["nhi16"],
           tuple(sch["K_lo"]), tuple(sch["K_hi"]))
    if key not in _CACHE:
        _CACHE.clear()
        _CACHE[key] = _build_kernel(sch)
    nc = _CACHE[key]

    bct = bc.T.astype(np.float16).copy()          # [128, 3]
    bffnt = b_ffn[:, None].copy()                 # [128, 1]
    bfinr = np.tile(b_fin[None, :], (GPC, 1)).astype(np.float32)

    in_maps = []
    for c in range(N_CORES):
        in_maps.append({
            "xfm": sch["x_fm"][c],
            "wc": Wc.astype(np.float16), "bct": bct,
            "wffn": W_ffn, "bffnt": bffnt,
            "wfin": W_fin, "bfinr": bfinr,
            "idxlo": sch["idx_lo"][c], "idxhi": sch["idx_hi"][c],
            "ohmask": sch["ohmask"][c], "discol": sch["discol"][c],
            "diagdis": sch["diagdis"][c],
            "invc": sch["invcnt_rep"][c],
        })

    _CACHE["in_maps"] = in_maps
    res = run_bass_kernel_spmd(nc, in_maps, core_ids=list(range(N_CORES)))
    out = np.concatenate([res.results[c]["out"] for c in range(N_CORES)], 0)
    return out.astype(np.float32)


def timed_run(inputs=None):
    """Re-run the cached compiled kernel with profiling; returns exec ns."""
    import time
    nc = next(v for k, v in _CACHE.items() if k != "in_maps")
    in_maps = _CACHE["in_maps"]
    walls = []
    for _ in range(3):
        t0 = time.time()
        run_bass_kernel_spmd(nc, in_maps, core_ids=list(range(N_CORES)))
        walls.append(time.time() - t0)
    print(f"warm re-run walls: {[f'{w*1e3:.1f}ms' for w in walls]}")
    try:
        res = run_bass_kernel_spmd(
            nc, in_maps, core_ids=list(range(N_CORES)), trace=True)
        if res.exec_time_ns is not None:
            return res.exec_time_ns
    except Exception as e:
        print(f"(ntff profiling unavailable: {type(e).__name__}: {e}; "
              f"reporting warm wall-clock upper bound)")
    return int(min(walls) * 1e9)


if __name__ == "__main__":
    rng = np.random.default_rng(0)
    x = rng.standard_normal((N_NODES, D), dtype=np.float32)
    ei = rng.integers(0, N_NODES, (2, N_EDGES)).astype(np.int64)
    batch = np.sort(rng.integers(0, N_GRAPHS, N_NODES)).astype(np.int64)
    Wc = rng.standard_normal((3, D, D), dtype=np.float32) * 0.05
    out = kernel(x, Wc, np.zeros((3, D), np.float32),
                 rng.standard_normal((2 * D, D), dtype=np.float32) * 0.05,
                 np.zeros((D,), np.float32),
                 rng.standard_normal((D, 2), dtype=np.float32) * 0.05,
                 np.zeros((2,), np.float32), ei, batch)
    print(out.shape, out[:4])
